# revision 10
# baseline (speedup 1.0000x reference)
"""Chamfer distance (pytorch3d defaults) on 8 Trainium2 NeuronCores.

Problem: gts_X, pred_X: [4, 8192, 3] fp32. loss = mean_b mean_n min_p d(x_bn, y_bp)
                                              + mean_b mean_p min_n d(x_bn, y_bp),
d = squared euclidean distance. gts_normals is unused (reference default path).

Sharding: 8 independent tasks = 4 batches x 2 directions, one per core.
Each core computes per-query windowed min over a 1024-wide, per-row-block
centered window of z-sorted refs; the host certifies each query with a z-gap
guard and recomputes the uncertified queries exactly in numpy.

Device algorithm per core (v2c):
- d[q, r] = |Q|^2 + |R|^2 - 2 Q.R via ONE K=16 bf16 matmul per (128q x 512r)
  tile using an exact hi/lo bf16 split (~fp32 precision in PSUM). Matmuls are
  packed 4x with tile_position row groups (keeps the PE at the 267ns/tile
  fused-weight-load pace; unpacked they cost 618+134ns).
- Per 128-query row block m: window = refs [lo_m, lo_m+1024) -> 2 matmuls
  into a [128, 2, 512] PSUM tile.
- PSUM drain (the wall: only DVE and ACT can read PSUM, ~1 elem/cycle/lane):
  - S-blocks (even m): ONE ACT op: out=exp(-BETA*d) with accum_out giving
    S_q = sum_r exp(-BETA * d_qr); the host recovers the windowed softmin
    -ln(S)/BETA (bias ~ -1e-5, validated under the 2e-2 tolerance; S==0 /
    tiny-S queries are recomputed exactly on host, as are guard escapes).
  - D-blocks (odd m): ONE DVE tensor_reduce XY straight off PSUM -> exact min.
  Each engine drains half the elements with zero cross-engine coupling.
"""

import sys

sys.path.insert(0, "/opt/trn_rl_repo")

import numpy as np
import ml_dtypes

import concourse.bacc as bacc
import concourse.mybir as mybir
from concourse.tile import TileContext
from concourse.bass_utils import run_bass_kernel_spmd

BF16 = ml_dtypes.bfloat16

B = 4
N = 8192
K = 13  # contraction rows after hi/lo split (ll cross term dropped)
MBLK = 128  # queries per row block (PSUM partitions)
NBLK = 512  # refs per matmul (one PSUM bank of fp32)
NMB = N // MBLK  # 64 row blocks
SB = NMB // 4  # 16 super-blocks of 4 row blocks
TAIL = 8  # blocks on each end that scan half-width windows
WS = [512 if (m < TAIL or m >= NMB - TAIL) else 768 for m in range(NMB)]

# per-row-block window start (centered on the block's rank range)
LOS = [min(max(128 * m + 64 - WS[m] // 2, 0), N - WS[m]) for m in range(NMB)]

BETA = 2500.0  # softmin sharpness (squared-distance units)
S_MIN = float(np.exp(-75.0))  # below this the softmin is underflow-suspect


def _is_soft(m):
    return m % 2 == 0 and m not in (28, 36)


LAST_RESULTS = None  # BassKernelResults of the most recent run (for test.py)


def _build_bass():
    nc = bacc.Bacc("TRN2")
    lhs = nc.dram_tensor("lhs", [K, N], mybir.dt.bfloat16, kind="ExternalInput")
    rhs = nc.dram_tensor("rhs", [K, N], mybir.dt.bfloat16, kind="ExternalInput")
    n_s = sum(_is_soft(m) for m in range(NMB))
    out_s = nc.dram_tensor("out_s", [MBLK, n_s], mybir.dt.float32, kind="ExternalOutput")
    out_d = nc.dram_tensor("out_d", [MBLK, NMB - n_s], mybir.dt.float32, kind="ExternalOutput")
    mn = mybir.AluOpType.min

    with TileContext(nc) as tc:
        with (
            tc.tile_pool(name="data", bufs=1) as data_pool,
            tc.tile_pool(name="work", bufs=4) as work_pool,
            tc.tile_pool(name="ps", bufs=4, space="PSUM") as ps_pool,
        ):
            # operands replicated at partition offsets 0/32 (separate tiles,
            # column-chunked DMAs) so adjacent blocks' matmuls overlap in the
            # PE array and the first blocks start after ~1/16 of the input DMA
            l0 = data_pool.tile([K, N], mybir.dt.bfloat16)
            r0 = data_pool.tile([K, N], mybir.dt.bfloat16)
            l1 = data_pool.tile([32 + K, N], mybir.dt.bfloat16)
            r1 = data_pool.tile([32 + K, N], mybir.dt.bfloat16)
            CH = N // 8
            for c in range(8):
                cs = slice(c * CH, (c + 1) * CH)
                nc.gpsimd.dma_start(l0[:, cs], lhs.ap()[:, cs])
                nc.gpsimd.dma_start(r0[:, cs], rhs.ap()[:, cs])
            for c in range(8):
                cs = slice(c * CH, (c + 1) * CH)
                nc.gpsimd.dma_start(l1[32 : 32 + K, cs], lhs.ap()[:, cs])
                nc.gpsimd.dma_start(r1[32 : 32 + K, cs], rhs.ap()[:, cs])
            lrep = [l0, l1]
            rrep = [r0, r1]

            arena_s = data_pool.tile([MBLK, n_s], mybir.dt.float32)
            arena_d = data_pool.tile([MBLK, NMB - n_s], mybir.dt.float32)
            i_s = i_d = 0

            for s in range(SB):
                for j in range(4):
                    m = 4 * s + j
                    g = m % 2
                    po = 32 * g
                    lt, rt = lrep[g], rrep[g]
                    lo = LOS[m]
                    w = WS[m]
                    ps = ps_pool.tile([MBLK, 2, NBLK], mybir.dt.float32, tag="ps")
                    psf = ps[:].rearrange("p a b -> p (a b)")
                    off = 0
                    while off < w:
                        cw = min(NBLK - off % NBLK, w - off)
                        nc.tensor.matmul(
                            psf[:, off : off + cw],
                            lt[po : po + K, m * MBLK : (m + 1) * MBLK],
                            rt[po : po + K, lo + off : lo + off + cw],
                            start=True,
                            stop=True,
                            tile_position=(po, 0),
                        )
                        off += cw
                    if _is_soft(m):  # S-block: ACT softmin (exp + sum-accum)
                        scratch = work_pool.tile(
                            [MBLK, 2, NBLK], mybir.dt.bfloat16, tag="sc"
                        )
                        nc.scalar.activation(
                            scratch[:].rearrange("p a b -> p (a b)")[:, 0:w],
                            psf[:, 0:w],
                            mybir.ActivationFunctionType.Exp,
                            bias=0.0,
                            scale=-BETA,
                            accum_out=arena_s[:, i_s : i_s + 1],
                        )
                        i_s += 1
                    else:  # D-block: DVE exact min straight off PSUM
                        nc.vector.tensor_reduce(
                            arena_d[:, i_d : i_d + 1],
                            psf[:, 0:w],
                            axis=mybir.AxisListType.X,
                            op=mn,
                        )
                        i_d += 1

            nc.sync.dma_start(out_s.ap(), arena_s[:])
            nc.sync.dma_start(out_d.ap(), arena_d[:])
    return nc


def _split_bf16(v):
    """v (fp32) ~= hi + lo with both bf16; residual is O(2^-18 |v|)."""
    hi = v.astype(BF16)
    lo = (v - hi.astype(np.float32)).astype(BF16)
    return hi, lo


def _prep_core_inputs(Q, R):
    """Build the K=16 lhsT (queries) and rhs (refs) bf16 matrices so that
    lhsT.T @ rhs accumulated in fp32 equals |Q|^2 + |R|^2 - 2 Q.R."""
    Qh, Ql = _split_bf16(Q)  # [N, 3]
    Rh, Rl = _split_bf16(-2.0 * R)  # [N, 3]
    nQh, nQl = _split_bf16((Q * Q).sum(axis=1))  # [N]
    nRh, nRl = _split_bf16((R * R).sum(axis=1))  # [N]
    one = np.ones(N, dtype=BF16)

    L = np.empty([K, N], dtype=BF16)
    L[0:3] = Qh.T
    L[3:6] = Qh.T
    L[6:9] = Ql.T
    L[9] = nQh
    L[10] = nQl
    L[11] = one
    L[12] = one

    Rm = np.empty([K, N], dtype=BF16)
    Rm[0:3] = Rh.T
    Rm[3:6] = Rl.T
    Rm[6:9] = Rh.T
    Rm[9] = one
    Rm[10] = one
    Rm[11] = nRh
    Rm[12] = nRl
    return L, Rm


def _try_axon_reset():
    """The axon-tunneled device sporadically wedges (NRT_EXEC_UNIT_UNRECOVERABLE);
    axon_reset() recovers it."""
    try:
        import ctypes

        import jax

        jax.devices()
        lib = ctypes.CDLL("/opt/axon/libaxon_pjrt.so")
        lib.axon_reset.restype = ctypes.c_int64
        lib.axon_reset()
    except Exception:
        pass


def _task_pairs(gts_X, pred_X):
    for b in range(B):
        yield gts_X[b], pred_X[b]  # each gts point -> nearest pred
        yield pred_X[b], gts_X[b]  # each pred point -> nearest gts


def kernel(gts_X, pred_X, gts_normals=None, **_ignored):
    global LAST_RESULTS
    gts_X = np.asarray(gts_X, dtype=np.float32)
    pred_X = np.asarray(pred_X, dtype=np.float32)
    assert gts_X.shape == (B, N, 3) and pred_X.shape == (B, N, 3)

    in_maps = []
    sorted_pairs = []
    for Qr, Rr in _task_pairs(gts_X, pred_X):
        Qs = np.ascontiguousarray(Qr[np.argsort(Qr[:, 2], kind="stable")])
        Rs = np.ascontiguousarray(Rr[np.argsort(Rr[:, 2], kind="stable")])
        sorted_pairs.append((Qs, Rs))
        L, Rm = _prep_core_inputs(Qs, Rs)
        in_maps.append({"lhs": L, "rhs": Rm})

    nc = _build_bass()
    nc.finalize()
    res = None
    for attempt in range(3):
        try:
            res = run_bass_kernel_spmd(nc, in_maps, core_ids=list(range(8)))
            break
        except Exception:
            if attempt == 2:
                raise
            _try_axon_reset()
    LAST_RESULTS = res

    los = np.array(LOS)
    q_idx = np.arange(N)
    lo = los[q_idx // MBLK]  # per-query window start
    hi = lo + np.array(WS)[q_idx // MBLK]
    soft = np.array([_is_soft(m) for m in range(NMB)])[q_idx // MBLK]
    s_blocks = [m for m in range(NMB) if _is_soft(m)]
    d_blocks = [m for m in range(NMB) if not _is_soft(m)]

    total = 0.0
    for (Qs, Rs), r in zip(sorted_pairs, res.results):
        vals = np.empty((NMB, MBLK))  # [block, partition]; query rank = m*128+p
        vals[s_blocks] = r["out_s"].astype(np.float64).T
        vals[d_blocks] = r["out_d"].astype(np.float64).T
        vals = vals.reshape(-1)
        mins = np.where(
            soft,
            -np.log(np.maximum(vals, 1e-300)) / BETA,  # softmin recovery
            vals,
        )
        # certification: true NN outside the window only if the squared z-gap
        # to the window edge is below the windowed min (pad for softmin bias /
        # exp-table error); softmin underflow (tiny S) is also uncertified.
        zq = Qs[:, 2].astype(np.float64)
        zr = Rs[:, 2].astype(np.float64)
        gap_lo = np.where(lo > 0, zq - zr[np.maximum(lo - 1, 0)], np.inf)
        gap_hi = np.where(hi < N, zr[np.minimum(hi, N - 1)] - zq, np.inf)
        guard = np.minimum(gap_lo, gap_hi) ** 2
        bad = (mins > guard * (1.0 - 2.0**-7)) | (soft & (vals < S_MIN))
        bad = np.nonzero(bad)[0]
        if len(bad):
            Qb = Qs[bad].astype(np.float64)
            Rd = Rs.astype(np.float64)
            nq = (Qb * Qb).sum(1)
            nr = (Rd * Rd).sum(1)
            d = nq[:, None] + nr[None, :] - 2.0 * (Qb @ Rd.T)
            mins[bad] = d.min(axis=1)
        total += mins.sum()

    loss = total / (B * N)
    return np.asarray(loss, dtype=np.float32)


# revision 11
# speedup vs baseline: 1.3108x; 1.3108x over previous
"""Chamfer distance (pytorch3d defaults) on 8 Trainium2 NeuronCores.

Problem: gts_X, pred_X: [4, 8192, 3] fp32. loss = mean_b mean_n min_p d(x_bn, y_bp)
                                              + mean_b mean_p min_n d(x_bn, y_bp),
d = squared euclidean distance. gts_normals is unused (reference default path).

Sharding: 8 independent tasks = 4 batches x 2 directions, one per core.
Each core computes per-query windowed min over a 1024-wide, per-row-block
centered window of z-sorted refs; the host certifies each query with a z-gap
guard and recomputes the uncertified queries exactly in numpy.

Device algorithm per core (v2c):
- d[q, r] = |Q|^2 + |R|^2 - 2 Q.R via ONE K=16 bf16 matmul per (128q x 512r)
  tile using an exact hi/lo bf16 split (~fp32 precision in PSUM). Matmuls are
  packed 4x with tile_position row groups (keeps the PE at the 267ns/tile
  fused-weight-load pace; unpacked they cost 618+134ns).
- Per 128-query row block m: window = refs [lo_m, lo_m+1024) -> 2 matmuls
  into a [128, 2, 512] PSUM tile.
- PSUM drain (the wall: only DVE and ACT can read PSUM, ~1 elem/cycle/lane):
  - S-blocks (even m): ONE ACT op: out=exp(-BETA*d) with accum_out giving
    S_q = sum_r exp(-BETA * d_qr); the host recovers the windowed softmin
    -ln(S)/BETA (bias ~ -1e-5, validated under the 2e-2 tolerance; S==0 /
    tiny-S queries are recomputed exactly on host, as are guard escapes).
  - D-blocks (odd m): ONE DVE tensor_reduce XY straight off PSUM -> exact min.
  Each engine drains half the elements with zero cross-engine coupling.
"""

import sys

sys.path.insert(0, "/opt/trn_rl_repo")

import numpy as np
import ml_dtypes

import concourse.bacc as bacc
import concourse.mybir as mybir
from concourse.tile import TileContext
from concourse.bass_utils import run_bass_kernel_spmd

BF16 = ml_dtypes.bfloat16

B = 4
N = 8192
K = 13  # contraction rows after hi/lo split (ll cross term dropped)
MBLK = 128  # queries per row block (PSUM partitions)
NBLK = 512  # refs per matmul (one PSUM bank of fp32)
NMB = N // MBLK  # 64 row blocks
SB = NMB // 4  # 16 super-blocks of 4 row blocks
TAIL = 8  # blocks on each end that scan half-width windows
WS = [512 if (m < TAIL or m >= NMB - TAIL) else 768 for m in range(NMB)]

# per-row-block window start (centered on the block's rank range)
LOS = [min(max(128 * m + 64 - WS[m] // 2, 0), N - WS[m]) for m in range(NMB)]

BETA = 2500.0  # softmin sharpness (squared-distance units)
S_MIN = float(np.exp(-75.0))  # below this the softmin is underflow-suspect


def _is_soft(m):
    return m % 2 == 0 and m not in (28, 36)


LAST_RESULTS = None  # BassKernelResults of the most recent run (for test.py)


def _build_bass():
    nc = bacc.Bacc("TRN2")
    lhs = nc.dram_tensor("lhs", [K, N], mybir.dt.bfloat16, kind="ExternalInput")
    rhs = nc.dram_tensor("rhs", [K, N], mybir.dt.bfloat16, kind="ExternalInput")
    n_s = sum(_is_soft(m) for m in range(NMB))
    out_s = nc.dram_tensor("out_s", [MBLK, n_s], mybir.dt.float32, kind="ExternalOutput")
    out_d = nc.dram_tensor("out_d", [MBLK, NMB - n_s], mybir.dt.float32, kind="ExternalOutput")
    mn = mybir.AluOpType.min

    with TileContext(nc) as tc:
        with (
            tc.tile_pool(name="data", bufs=1) as data_pool,
            tc.tile_pool(name="work", bufs=4) as work_pool,
            tc.tile_pool(name="ps", bufs=4, space="PSUM") as ps_pool,
        ):
            # operands replicated at partition offsets 0/32 (separate tiles,
            # column-chunked DMAs) so adjacent blocks' matmuls overlap in the
            # PE array and the first blocks start after ~1/16 of the input DMA
            # prefix tiles: first 8 blocks' operand columns, DMA'd first so
            # the PE starts ~6us earlier than the full-tile loads allow
            PFX = 1664
            l0a = data_pool.tile([K, PFX], mybir.dt.bfloat16)
            r0a = data_pool.tile([K, PFX], mybir.dt.bfloat16)
            l1a = data_pool.tile([32 + K, PFX], mybir.dt.bfloat16)
            r1a = data_pool.tile([32 + K, PFX], mybir.dt.bfloat16)
            nc.sync.dma_start(l0a[:], lhs.ap()[:, 0:PFX])
            nc.sync.dma_start(r0a[:], rhs.ap()[:, 0:PFX])
            nc.sync.dma_start(l1a[32 : 32 + K, :], lhs.ap()[:, 0:PFX])
            nc.sync.dma_start(r1a[32 : 32 + K, :], rhs.ap()[:, 0:PFX])
            l0 = data_pool.tile([K, N], mybir.dt.bfloat16)
            r0 = data_pool.tile([K, N], mybir.dt.bfloat16)
            l1 = data_pool.tile([32 + K, N], mybir.dt.bfloat16)
            r1 = data_pool.tile([32 + K, N], mybir.dt.bfloat16)
            CH = N // 8
            for c in range(8):
                cs = slice(c * CH, (c + 1) * CH)
                nc.sync.dma_start(l0[:, cs], lhs.ap()[:, cs])
                nc.sync.dma_start(r0[:, cs], rhs.ap()[:, cs])
                nc.sync.dma_start(l1[32 : 32 + K, cs], lhs.ap()[:, cs])
                nc.sync.dma_start(r1[32 : 32 + K, cs], rhs.ap()[:, cs])
            lrep = [l0, l1]
            rrep = [r0, r1]
            lrep_a = [l0a, l1a]
            rrep_a = [r0a, r1a]

            arena_s = data_pool.tile([MBLK, n_s], mybir.dt.float32)
            arena_d = data_pool.tile([MBLK, NMB - n_s], mybir.dt.float32)
            i_s = i_d = 0

            for s in range(SB):
                for j in range(4):
                    m = 4 * s + j
                    g = m % 2
                    po = 32 * g
                    if m < 8:  # prefix tiles cover cols < PFX
                        lt, rt = lrep_a[g], rrep_a[g]
                    else:
                        lt, rt = lrep[g], rrep[g]
                    lo = LOS[m]
                    w = WS[m]
                    ps = ps_pool.tile([MBLK, 2, NBLK], mybir.dt.float32, tag="ps")
                    psf = ps[:].rearrange("p a b -> p (a b)")
                    off = 0
                    while off < w:
                        cw = min(NBLK - off % NBLK, w - off)
                        nc.tensor.matmul(
                            psf[:, off : off + cw],
                            lt[po : po + K, m * MBLK : (m + 1) * MBLK],
                            rt[po : po + K, lo + off : lo + off + cw],
                            start=True,
                            stop=True,
                            tile_position=(po, 0),
                        )
                        off += cw
                    if _is_soft(m):  # S-block: ACT softmin (exp + sum-accum)
                        scratch = work_pool.tile(
                            [MBLK, 2, NBLK], mybir.dt.bfloat16, tag="sc"
                        )
                        nc.scalar.activation(
                            scratch[:].rearrange("p a b -> p (a b)")[:, 0:w],
                            psf[:, 0:w],
                            mybir.ActivationFunctionType.Exp,
                            bias=0.0,
                            scale=-BETA,
                            accum_out=arena_s[:, i_s : i_s + 1],
                        )
                        i_s += 1
                    else:  # D-block: DVE exact min straight off PSUM
                        nc.vector.tensor_reduce(
                            arena_d[:, i_d : i_d + 1],
                            psf[:, 0:w],
                            axis=mybir.AxisListType.X,
                            op=mn,
                        )
                        i_d += 1

            nc.sync.dma_start(out_s.ap(), arena_s[:])
            nc.sync.dma_start(out_d.ap(), arena_d[:])
    return nc


def _split_bf16(v):
    """v (fp32) ~= hi + lo with both bf16; residual is O(2^-18 |v|)."""
    hi = v.astype(BF16)
    lo = (v - hi.astype(np.float32)).astype(BF16)
    return hi, lo


def _prep_core_inputs(Q, R):
    """Build the K=16 lhsT (queries) and rhs (refs) bf16 matrices so that
    lhsT.T @ rhs accumulated in fp32 equals |Q|^2 + |R|^2 - 2 Q.R."""
    Qh, Ql = _split_bf16(Q)  # [N, 3]
    Rh, Rl = _split_bf16(-2.0 * R)  # [N, 3]
    nQh, nQl = _split_bf16((Q * Q).sum(axis=1))  # [N]
    nRh, nRl = _split_bf16((R * R).sum(axis=1))  # [N]
    one = np.ones(N, dtype=BF16)

    L = np.empty([K, N], dtype=BF16)
    L[0:3] = Qh.T
    L[3:6] = Qh.T
    L[6:9] = Ql.T
    L[9] = nQh
    L[10] = nQl
    L[11] = one
    L[12] = one

    Rm = np.empty([K, N], dtype=BF16)
    Rm[0:3] = Rh.T
    Rm[3:6] = Rl.T
    Rm[6:9] = Rh.T
    Rm[9] = one
    Rm[10] = one
    Rm[11] = nRh
    Rm[12] = nRl
    return L, Rm


def _try_axon_reset():
    """The axon-tunneled device sporadically wedges (NRT_EXEC_UNIT_UNRECOVERABLE);
    axon_reset() recovers it."""
    try:
        import ctypes

        import jax

        jax.devices()
        lib = ctypes.CDLL("/opt/axon/libaxon_pjrt.so")
        lib.axon_reset.restype = ctypes.c_int64
        lib.axon_reset()
    except Exception:
        pass


def _task_pairs(gts_X, pred_X):
    for b in range(B):
        yield gts_X[b], pred_X[b]  # each gts point -> nearest pred
        yield pred_X[b], gts_X[b]  # each pred point -> nearest gts


def kernel(gts_X, pred_X, gts_normals=None, **_ignored):
    global LAST_RESULTS
    gts_X = np.asarray(gts_X, dtype=np.float32)
    pred_X = np.asarray(pred_X, dtype=np.float32)
    assert gts_X.shape == (B, N, 3) and pred_X.shape == (B, N, 3)

    in_maps = []
    sorted_pairs = []
    for Qr, Rr in _task_pairs(gts_X, pred_X):
        Qs = np.ascontiguousarray(Qr[np.argsort(Qr[:, 2], kind="stable")])
        Rs = np.ascontiguousarray(Rr[np.argsort(Rr[:, 2], kind="stable")])
        sorted_pairs.append((Qs, Rs))
        L, Rm = _prep_core_inputs(Qs, Rs)
        in_maps.append({"lhs": L, "rhs": Rm})

    nc = _build_bass()
    nc.finalize()
    res = None
    for attempt in range(3):
        try:
            res = run_bass_kernel_spmd(nc, in_maps, core_ids=list(range(8)))
            break
        except Exception:
            if attempt == 2:
                raise
            _try_axon_reset()
    LAST_RESULTS = res

    los = np.array(LOS)
    q_idx = np.arange(N)
    lo = los[q_idx // MBLK]  # per-query window start
    hi = lo + np.array(WS)[q_idx // MBLK]
    soft = np.array([_is_soft(m) for m in range(NMB)])[q_idx // MBLK]
    s_blocks = [m for m in range(NMB) if _is_soft(m)]
    d_blocks = [m for m in range(NMB) if not _is_soft(m)]

    total = 0.0
    for (Qs, Rs), r in zip(sorted_pairs, res.results):
        vals = np.empty((NMB, MBLK))  # [block, partition]; query rank = m*128+p
        vals[s_blocks] = r["out_s"].astype(np.float64).T
        vals[d_blocks] = r["out_d"].astype(np.float64).T
        vals = vals.reshape(-1)
        mins = np.where(
            soft,
            -np.log(np.maximum(vals, 1e-300)) / BETA,  # softmin recovery
            vals,
        )
        # certification: true NN outside the window only if the squared z-gap
        # to the window edge is below the windowed min (pad for softmin bias /
        # exp-table error); softmin underflow (tiny S) is also uncertified.
        zq = Qs[:, 2].astype(np.float64)
        zr = Rs[:, 2].astype(np.float64)
        gap_lo = np.where(lo > 0, zq - zr[np.maximum(lo - 1, 0)], np.inf)
        gap_hi = np.where(hi < N, zr[np.minimum(hi, N - 1)] - zq, np.inf)
        guard = np.minimum(gap_lo, gap_hi) ** 2
        bad = (mins > guard * (1.0 - 2.0**-7)) | (soft & (vals < S_MIN))
        bad = np.nonzero(bad)[0]
        if len(bad):
            Qb = Qs[bad].astype(np.float64)
            Rd = Rs.astype(np.float64)
            nq = (Qb * Qb).sum(1)
            nr = (Rd * Rd).sum(1)
            d = nq[:, None] + nr[None, :] - 2.0 * (Qb @ Rd.T)
            mins[bad] = d.min(axis=1)
        total += mins.sum()

    loss = total / (B * N)
    return np.asarray(loss, dtype=np.float32)


# revision 12
# speedup vs baseline: 1.3802x; 1.0530x over previous
"""Chamfer distance (pytorch3d defaults) on 8 Trainium2 NeuronCores.

Problem: gts_X, pred_X: [4, 8192, 3] fp32. loss = mean_b mean_n min_p d(x_bn, y_bp)
                                              + mean_b mean_p min_n d(x_bn, y_bp),
d = squared euclidean distance. gts_normals is unused (reference default path).

Sharding: 8 independent tasks = 4 batches x 2 directions, one per core.
Each core computes per-query windowed min over a 1024-wide, per-row-block
centered window of z-sorted refs; the host certifies each query with a z-gap
guard and recomputes the uncertified queries exactly in numpy.

Device algorithm per core (v2c):
- d[q, r] = |Q|^2 + |R|^2 - 2 Q.R via ONE K=16 bf16 matmul per (128q x 512r)
  tile using an exact hi/lo bf16 split (~fp32 precision in PSUM). Matmuls are
  packed 4x with tile_position row groups (keeps the PE at the 267ns/tile
  fused-weight-load pace; unpacked they cost 618+134ns).
- Per 128-query row block m: window = refs [lo_m, lo_m+1024) -> 2 matmuls
  into a [128, 2, 512] PSUM tile.
- PSUM drain (the wall: only DVE and ACT can read PSUM, ~1 elem/cycle/lane):
  - S-blocks (even m): ONE ACT op: out=exp(-BETA*d) with accum_out giving
    S_q = sum_r exp(-BETA * d_qr); the host recovers the windowed softmin
    -ln(S)/BETA (bias ~ -1e-5, validated under the 2e-2 tolerance; S==0 /
    tiny-S queries are recomputed exactly on host, as are guard escapes).
  - D-blocks (odd m): ONE DVE tensor_reduce XY straight off PSUM -> exact min.
  Each engine drains half the elements with zero cross-engine coupling.
"""

import sys

sys.path.insert(0, "/opt/trn_rl_repo")

import numpy as np
import ml_dtypes

import concourse.bacc as bacc
import concourse.mybir as mybir
from concourse.tile import TileContext
from concourse.bass_utils import run_bass_kernel_spmd

BF16 = ml_dtypes.bfloat16

B = 4
N = 8192
K = 13  # contraction rows after hi/lo split (ll cross term dropped)
MBLK = 128  # queries per row block (PSUM partitions)
NBLK = 512  # refs per matmul (one PSUM bank of fp32)
NMB = N // MBLK  # 64 row blocks
SB = NMB // 4  # 16 super-blocks of 4 row blocks
TAIL = 8  # blocks on each end that scan half-width windows
WS = [512 if (m < TAIL or m >= NMB - TAIL) else 768 for m in range(NMB)]

# per-row-block window start (centered on the block's rank range)
LOS = [min(max(128 * m + 64 - WS[m] // 2, 0), N - WS[m]) for m in range(NMB)]

BETA = 2500.0  # softmin sharpness (squared-distance units)
S_MIN = float(np.exp(-75.0))  # below this the softmin is underflow-suspect


def _is_soft(m):
    return m % 2 == 0 and m not in (28, 36)


LAST_RESULTS = None  # BassKernelResults of the most recent run (for test.py)


def _build_bass():
    nc = bacc.Bacc("TRN2")
    lhs = nc.dram_tensor("lhs", [K, N], mybir.dt.bfloat16, kind="ExternalInput")
    rhs = nc.dram_tensor("rhs", [K, N], mybir.dt.bfloat16, kind="ExternalInput")
    n_s = sum(_is_soft(m) for m in range(NMB))
    out_s = nc.dram_tensor("out_s", [MBLK, n_s], mybir.dt.float32, kind="ExternalOutput")
    out_d = nc.dram_tensor("out_d", [MBLK, NMB - n_s], mybir.dt.float32, kind="ExternalOutput")
    mn = mybir.AluOpType.min

    with TileContext(nc) as tc:
        with (
            tc.tile_pool(name="data", bufs=1) as data_pool,
            tc.tile_pool(name="work", bufs=4) as work_pool,
            tc.tile_pool(name="ps", bufs=4, space="PSUM") as ps_pool,
        ):
            # operands replicated at partition offsets 0/32 (separate tiles,
            # column-chunked DMAs) so adjacent blocks' matmuls overlap in the
            # PE array and the first blocks start after ~1/16 of the input DMA
            l0 = data_pool.tile([K, N], mybir.dt.bfloat16)
            r0 = data_pool.tile([K, N], mybir.dt.bfloat16)
            l1 = data_pool.tile([32 + K, N], mybir.dt.bfloat16)
            r1 = data_pool.tile([32 + K, N], mybir.dt.bfloat16)
            CH = N // 8
            for c in range(8):
                cs = slice(c * CH, (c + 1) * CH)
                nc.sync.dma_start(l0[:, cs], lhs.ap()[:, cs])
                nc.sync.dma_start(r0[:, cs], rhs.ap()[:, cs])
                nc.sync.dma_start(l1[32 : 32 + K, cs], lhs.ap()[:, cs])
                nc.sync.dma_start(r1[32 : 32 + K, cs], rhs.ap()[:, cs])
            lrep = [l0, l1]
            rrep = [r0, r1]

            arena_s = data_pool.tile([MBLK, n_s], mybir.dt.float32)
            arena_d = data_pool.tile([MBLK, NMB - n_s], mybir.dt.float32)
            i_s = i_d = 0

            for s in range(SB):
                for j in range(4):
                    m = 4 * s + j
                    g = m % 2
                    po = 32 * g
                    lt, rt = lrep[g], rrep[g]
                    lo = LOS[m]
                    w = WS[m]
                    ps = ps_pool.tile([MBLK, 2, NBLK], mybir.dt.float32, tag="ps")
                    psf = ps[:].rearrange("p a b -> p (a b)")
                    off = 0
                    while off < w:
                        cw = min(NBLK - off % NBLK, w - off)
                        nc.tensor.matmul(
                            psf[:, off : off + cw],
                            lt[po : po + K, m * MBLK : (m + 1) * MBLK],
                            rt[po : po + K, lo + off : lo + off + cw],
                            start=True,
                            stop=True,
                            tile_position=(po, 0),
                        )
                        off += cw
                    if _is_soft(m):  # S-block: ACT softmin (exp + sum-accum)
                        scratch = work_pool.tile(
                            [MBLK, 2, NBLK], mybir.dt.bfloat16, tag="sc"
                        )
                        nc.scalar.activation(
                            scratch[:].rearrange("p a b -> p (a b)")[:, 0:w],
                            psf[:, 0:w],
                            mybir.ActivationFunctionType.Exp,
                            bias=0.0,
                            scale=-BETA,
                            accum_out=arena_s[:, i_s : i_s + 1],
                        )
                        i_s += 1
                    else:  # D-block: DVE exact min straight off PSUM
                        nc.vector.tensor_reduce(
                            arena_d[:, i_d : i_d + 1],
                            psf[:, 0:w],
                            axis=mybir.AxisListType.X,
                            op=mn,
                        )
                        i_d += 1

            nc.sync.dma_start(out_s.ap(), arena_s[:])
            nc.sync.dma_start(out_d.ap(), arena_d[:])
    return nc


def _split_bf16(v):
    """v (fp32) ~= hi + lo with both bf16; residual is O(2^-18 |v|)."""
    hi = v.astype(BF16)
    lo = (v - hi.astype(np.float32)).astype(BF16)
    return hi, lo


def _prep_core_inputs(Q, R):
    """Build the K=16 lhsT (queries) and rhs (refs) bf16 matrices so that
    lhsT.T @ rhs accumulated in fp32 equals |Q|^2 + |R|^2 - 2 Q.R."""
    Qh, Ql = _split_bf16(Q)  # [N, 3]
    Rh, Rl = _split_bf16(-2.0 * R)  # [N, 3]
    nQh, nQl = _split_bf16((Q * Q).sum(axis=1))  # [N]
    nRh, nRl = _split_bf16((R * R).sum(axis=1))  # [N]
    one = np.ones(N, dtype=BF16)

    L = np.empty([K, N], dtype=BF16)
    L[0:3] = Qh.T
    L[3:6] = Qh.T
    L[6:9] = Ql.T
    L[9] = nQh
    L[10] = nQl
    L[11] = one
    L[12] = one

    Rm = np.empty([K, N], dtype=BF16)
    Rm[0:3] = Rh.T
    Rm[3:6] = Rl.T
    Rm[6:9] = Rh.T
    Rm[9] = one
    Rm[10] = one
    Rm[11] = nRh
    Rm[12] = nRl
    return L, Rm


def _try_axon_reset():
    """The axon-tunneled device sporadically wedges (NRT_EXEC_UNIT_UNRECOVERABLE);
    axon_reset() recovers it."""
    try:
        import ctypes

        import jax

        jax.devices()
        lib = ctypes.CDLL("/opt/axon/libaxon_pjrt.so")
        lib.axon_reset.restype = ctypes.c_int64
        lib.axon_reset()
    except Exception:
        pass


def _task_pairs(gts_X, pred_X):
    for b in range(B):
        yield gts_X[b], pred_X[b]  # each gts point -> nearest pred
        yield pred_X[b], gts_X[b]  # each pred point -> nearest gts


def kernel(gts_X, pred_X, gts_normals=None, **_ignored):
    global LAST_RESULTS
    gts_X = np.asarray(gts_X, dtype=np.float32)
    pred_X = np.asarray(pred_X, dtype=np.float32)
    assert gts_X.shape == (B, N, 3) and pred_X.shape == (B, N, 3)

    in_maps = []
    sorted_pairs = []
    for Qr, Rr in _task_pairs(gts_X, pred_X):
        Qs = np.ascontiguousarray(Qr[np.argsort(Qr[:, 2], kind="stable")])
        Rs = np.ascontiguousarray(Rr[np.argsort(Rr[:, 2], kind="stable")])
        sorted_pairs.append((Qs, Rs))
        L, Rm = _prep_core_inputs(Qs, Rs)
        in_maps.append({"lhs": L, "rhs": Rm})

    nc = _build_bass()
    nc.finalize()
    res = None
    for attempt in range(3):
        try:
            res = run_bass_kernel_spmd(nc, in_maps, core_ids=list(range(8)))
            break
        except Exception:
            if attempt == 2:
                raise
            _try_axon_reset()
    LAST_RESULTS = res

    los = np.array(LOS)
    q_idx = np.arange(N)
    lo = los[q_idx // MBLK]  # per-query window start
    hi = lo + np.array(WS)[q_idx // MBLK]
    soft = np.array([_is_soft(m) for m in range(NMB)])[q_idx // MBLK]
    s_blocks = [m for m in range(NMB) if _is_soft(m)]
    d_blocks = [m for m in range(NMB) if not _is_soft(m)]

    total = 0.0
    for (Qs, Rs), r in zip(sorted_pairs, res.results):
        vals = np.empty((NMB, MBLK))  # [block, partition]; query rank = m*128+p
        vals[s_blocks] = r["out_s"].astype(np.float64).T
        vals[d_blocks] = r["out_d"].astype(np.float64).T
        vals = vals.reshape(-1)
        mins = np.where(
            soft,
            -np.log(np.maximum(vals, 1e-300)) / BETA,  # softmin recovery
            vals,
        )
        # certification: true NN outside the window only if the squared z-gap
        # to the window edge is below the windowed min (pad for softmin bias /
        # exp-table error); softmin underflow (tiny S) is also uncertified.
        zq = Qs[:, 2].astype(np.float64)
        zr = Rs[:, 2].astype(np.float64)
        gap_lo = np.where(lo > 0, zq - zr[np.maximum(lo - 1, 0)], np.inf)
        gap_hi = np.where(hi < N, zr[np.minimum(hi, N - 1)] - zq, np.inf)
        guard = np.minimum(gap_lo, gap_hi) ** 2
        bad = (mins > guard * (1.0 - 2.0**-7)) | (soft & (vals < S_MIN))
        bad = np.nonzero(bad)[0]
        if len(bad):
            Qb = Qs[bad].astype(np.float64)
            Rd = Rs.astype(np.float64)
            nq = (Qb * Qb).sum(1)
            nr = (Rd * Rd).sum(1)
            d = nq[:, None] + nr[None, :] - 2.0 * (Qb @ Rd.T)
            mins[bad] = d.min(axis=1)
        total += mins.sum()

    loss = total / (B * N)
    return np.asarray(loss, dtype=np.float32)
